# revision 3
# baseline (speedup 1.0000x reference)
"""Embedding lookup kernel for TRN2 (8 NeuronCores, vocab-sharded).

out[0, t, :] = W[:, idx[t]] + b   for t in [0, 32*8192)

Strategy (plan F): the host precomputes table = W.T + b in fp16 (rel err
~3e-4, far inside the 2e-2 gate) and shards the VOCAB across the 8
cores: core c owns rows [c*12500, (c+1)*12500) — a 3.2 MB slice — and
receives exactly the tokens whose index falls in its slice (one global
stable argsort groups them contiguously). The per-core gather footprint
stays inside 3.2 MB of HBM (row-buffer locality), the local row index
fits int16 directly (no windowing), and the dma_gather cost here is
per-DESCRIPTOR (~2 ns/desc measured, independent of 256 B vs 512 B
element size), so single-row 256 B descriptors halve the read bytes for
free and need no pair-select: zero DVE/compute work on the device.

Device per chunk (18 chunks round-robin over all 4 SWDGE queues): load
wrapped int16 row-indices (Act HWDGE queue), dma_gather 256 B fp16 rows
into SBUF ([p, s, d] holds sorted-list position p*spp+s), then one
contiguous dma_start (SP HWDGE) of the tile into the fp16 out buffer in
partition-major layout ([128, 34304] fp16; 3840 B per partition per
chunk on both the SBUF and DRAM side).

Token counts per core are multinomial(262144, 1/8): sigma ~169, so the
34304-slot cap is a +9 sigma bound. On overflow (adversarial index
distribution) fall back to plan A (replicated-table indirect-DMA
gather — slow but correct for any distribution).

Host packing transposes each chunk's index list so gather slot i =
list[(i%128)*spp + i//128]; host unpacking reshapes each chunk tile
back to list order, casts fp16->f32, and scatters rows to their token
positions (inverse of the global sort) — host-side unsharding glue.
"""

import numpy as np

import concourse.bacc as bacc
import concourse.mybir as mybir
import concourse.tile as tile
from concourse import bass
from concourse.bass_utils import run_bass_kernel_spmd

NCORES = 8
B, S = 32, 8192
TOKENS = B * S              # 262144
T = TOKENS // NCORES        # 32768 expected tokens per core
V = 100000
D = 128
VSH = V // NCORES           # 12500 vocab rows per core shard

CAP = 34304                 # padded token slots per core (+9 sigma)
CHUNK = 1920                # 15 rows per partition per chunk
CHUNKS = []                 # (device rowbase, cap)
_rb = 0
while _rb < CAP:
    CHUNKS.append((_rb, min(CHUNK, CAP - _rb)))
    _rb += CHUNKS[-1][1]
NCH = len(CHUNKS)
NQUEUES = 4

_compiled = {}


def _build(repeat=1, nqueues=NQUEUES):
    # repeat>1 replicates the body for repeat-slope timing (outputs just
    # get overwritten; timing only).
    nc = bacc.Bacc("TRN2", target_bir_lowering=False, debug=False,
                   num_swdge_queues=nqueues)
    idx16_d = nc.dram_tensor("idx16", [NCH, 128, CHUNK // 16], mybir.dt.int16,
                             kind="ExternalInput").ap()
    tab_d = nc.dram_tensor("tab", [VSH, D], mybir.dt.float16,
                           kind="ExternalInput").ap()
    out_d = nc.dram_tensor("out", [128, CAP], mybir.dt.float16,
                           kind="ExternalOutput").ap()

    with tile.TileContext(nc) as tc:
        with tc.tile_pool(name="idxp", bufs=8) as ip, \
             tc.tile_pool(name="pair", bufs=8) as pp:
            for _ in range(repeat):
                for ch, (rowbase, cap) in enumerate(CHUNKS):
                    it = ip.tile([128, cap // 16], mybir.dt.int16, tag="it")
                    nc.scalar.dma_start(out=it[:], in_=idx16_d[ch, :, :cap // 16])
                    pt = pp.tile([128, cap], mybir.dt.float16)
                    p3 = pt[:].rearrange("p (s e) -> p s e", e=D)
                    nc.gpsimd.dma_gather(
                        p3, tab_d, it[:],
                        num_idxs=cap, num_idxs_reg=cap, elem_size=D,
                        single_packet=False, queue_num=ch % nqueues)
                    nc.sync.dma_start(
                        out=out_d[:, rowbase:rowbase + cap], in_=pt[:])
    nc.compile()
    return nc


def _build_plan_a():
    G = 8
    NGATH = T // 128
    nc = bacc.Bacc("TRN2", target_bir_lowering=False, debug=False)
    idx_d = nc.dram_tensor("idx", [128, NGATH], mybir.dt.int32,
                           kind="ExternalInput").ap()
    tab_d = nc.dram_tensor("tab", [V, D], mybir.dt.float32,
                           kind="ExternalInput").ap()
    out_d = nc.dram_tensor("out", [T, D], mybir.dt.float32,
                           kind="ExternalOutput").ap()
    with tile.TileContext(nc) as tc:
        with tc.tile_pool(name="data", bufs=3) as dp, \
             tc.tile_pool(name="idxp", bufs=1) as ip:
            it = ip.tile([128, NGATH], mybir.dt.int32)
            nc.sync.dma_start(out=it[:], in_=idx_d[:])
            for c in range(T // (128 * G)):
                dt_ = dp.tile([128, G * D], mybir.dt.float32)
                for g in range(G):
                    nc.gpsimd.indirect_dma_start(
                        out=dt_[:, g * D:(g + 1) * D], out_offset=None,
                        in_=tab_d[:],
                        in_offset=bass.IndirectOffsetOnAxis(
                            ap=it[:, c * G + g:c * G + g + 1], axis=0),
                    )
                dst = out_d[c * G * 128:(c + 1) * G * 128, :] \
                    .rearrange("(g p) d -> p g d", p=128)
                nc.sync.dma_start(
                    out=dst, in_=dt_[:].rearrange("p (g d) -> p g d", g=G))
    nc.compile()
    return nc


def _get_nc(plan):
    if plan not in _compiled:
        _compiled[plan] = _build() if plan == "f" else _build_plan_a()
    return _compiled[plan]


def _wrap16(arr):
    # slot i -> partition i % 16, column i // 16; replicated to 128 partitions
    w = arr.reshape(-1, 16).T
    return np.ascontiguousarray(np.tile(w, (8, 1)))


def _pack_core(loc):
    """loc: [n] int32 core-local row indices (sorted) -> idx16."""
    n = loc.shape[0]
    full = np.full(CAP, loc[-1] if n else 0, np.int16)
    full[:n] = loc
    idx16 = np.zeros((NCH, 128, CHUNK // 16), np.int16)
    for ch, (rb, cap) in enumerate(CHUNKS):
        spp = cap // 128
        slots = full[rb:rb + cap].reshape(128, spp).T.reshape(-1)
        idx16[ch, :, :cap // 16] = _wrap16(slots)
    return idx16


def _make_in_maps(X, W, b):
    X = np.asarray(X)
    W = np.asarray(W, dtype=np.float32)
    b = np.asarray(b, dtype=np.float32)
    idx = np.ascontiguousarray(X.reshape(-1).astype(np.int32))
    table32 = np.ascontiguousarray(W.T) + b[None, :]
    table = table32.astype(np.float16)

    order = np.argsort(idx, kind="stable")
    sv = idx[order]
    bounds = np.searchsorted(sv, np.arange(NCORES + 1) * VSH)
    counts = np.diff(bounds)
    if counts.max() <= CAP:
        in_maps = []
        for c in range(NCORES):
            loc = sv[bounds[c]:bounds[c + 1]] - c * VSH
            in_maps.append({
                "idx16": _pack_core(loc),
                "tab": np.ascontiguousarray(table[c * VSH:(c + 1) * VSH]),
            })
        return "f", in_maps, (order, bounds)

    # shard-capacity overflow (pathological index distribution): plan A
    NGATH = T // 128
    in_maps = [
        {"idx": np.ascontiguousarray(
            idx[c * T:(c + 1) * T].reshape(NGATH, 128).T), "tab": table32}
        for c in range(NCORES)
    ]
    return "a", in_maps, None


def _unpack_f(res, meta):
    order, bounds = meta
    out = np.empty((TOKENS, D), np.float32)
    for c in range(NCORES):
        n = bounds[c + 1] - bounds[c]
        dev = np.asarray(res.results[c]["out"])          # [128, CAP] fp16
        rows = np.empty((CAP, D), np.float16)
        for ch, (rb, cap) in enumerate(CHUNKS):
            spp = cap // 128
            rows[rb:rb + cap] = dev[:, rb:rb + cap].reshape(128 * spp, D)
        out[order[bounds[c]:bounds[c + 1]]] = rows[:n].astype(np.float32)
    return out.reshape(1, TOKENS, D)


def kernel(X, W, b):
    plan, in_maps, meta = _make_in_maps(X, W, b)
    res = run_bass_kernel_spmd(_get_nc(plan), in_maps, list(range(NCORES)))
    if plan == "f":
        return _unpack_f(res, meta)
    out = np.concatenate(
        [res.results[c]["out"] for c in range(NCORES)], axis=0)
    return out.reshape(1, TOKENS, D)
